# revision 71
# baseline (speedup 1.0000x reference)
"""BaiLing attention Trainium2 kernel.

Sharding: 8 cores = 2 (batch) x 4 (tensor-parallel over heads).
Each TP rank r owns q heads 4r..4r+3 and kv head r (GQA group-aligned),
computes its out-projection partial; host sums the 4 partials per batch.

On-chip layout is feature-major [d, s] everywhere:
  - QKV proj:  psum[qkv_col, s] = w_qkv_slice.T @ hidden.T
  - scoresT:   psum[sk, sq]     = k_tile.T @ q_tile    (both [d, *])
  - causal mask: -1e9 accumulated into the scores psum via an
    identity-matmul of a constant mask tile
  - softmax over sk (partition dim): exp on ACT, denominator via
    ones-matmul, unnormalized PV; 1/x and 1/sqrt(x) of the [1,n]
    rows are computed as Exp(-c*Ln(x)) on ACT (DVE reciprocal on a
    single-partition row is lane-bound and ~25x slower) and broadcast
    across partitions with a K=1 ones-row matmul into PSUM (no DMA
    round-trips).
  - PV:        psum[d, sq]      = vT_tile.T @ probsT
  - out-proj:  psum[s, n]       = oT_slice.T @ w_o_slice
Matmuls run as float32r. Out-proj tiles are emitted one attention tile
late so the tensor engine queue never waits on the normalize chain.
"""

import sys

sys.path.insert(0, "/opt/trn_rl_repo")

import math
from contextlib import ExitStack

import ml_dtypes
import numpy as np

BF = ml_dtypes.bfloat16

import concourse.bass as bass
import concourse.mybir as mybir
import concourse.tile as tile
from concourse import bacc
from concourse.bass_utils import run_bass_kernel_spmd

F32 = mybir.dt.float32
F32R = mybir.dt.float32r
BF16 = mybir.dt.bfloat16
I32 = mybir.dt.int32
AF = mybir.ActivationFunctionType
OP = mybir.AluOpType

H = 2048          # hidden size
S = 2048          # sequence length
D = 128           # head dim
NH_L = 4          # q heads per rank
QW = NH_L * D     # 512 local q width
CW = QW + 2 * D   # 768 local qkv width
P = 128
KO = H // P       # 16 contraction tiles
ST = S // 512     # 4 seq tiles of 512
SM_SCALE = float(D) ** -0.5
EPS = 1e-6
ROPE_THETA = 10000.0
NEG = -1.0e9

# Cody-Waite split of 2*pi (short-mantissa terms: k*ci exact for k<=512)
CW1 = 6.28125
CW2 = 0.0019350051879882812
CW3 = 3.019916050561733e-07
PI = math.pi


def _build():
    nc = bacc.Bacc("TRN2", target_bir_lowering=False, debug=False, num_devices=8)

    hT = nc.dram_tensor("hT", [H, S], BF16, kind="ExternalInput").ap()
    wqkv = nc.dram_tensor("wqkv", [H, CW], BF16, kind="ExternalInput").ap()
    wo = nc.dram_tensor("wo", [QW, H], BF16, kind="ExternalInput").ap()
    cosbt = nc.dram_tensor("cosbt", [P, S], F32, kind="ExternalInput").ap()
    sinbt = nc.dram_tensor("sinbt", [P, S], F32, kind="ExternalInput").ap()
    wqn = nc.dram_tensor("wqn", [D, 1], F32, kind="ExternalInput").ap()
    wkn = nc.dram_tensor("wkn", [D, 1], F32, kind="ExternalInput").ap()
    maskneg = nc.dram_tensor("maskneg", [P, 896], BF16, kind="ExternalInput").ap()
    onesrow = nc.dram_tensor("onesrow", [1, P], BF16, kind="ExternalInput").ap()
    identr = nc.dram_tensor("identr", [P, P], BF16, kind="ExternalInput").ap()
    rmat = nc.dram_tensor("rmat", [P, P], BF16, kind="ExternalInput").ap()
    oh5 = nc.dram_tensor("oh5", [P, 5, 5], BF16, kind="ExternalInput").ap()
    oh4 = nc.dram_tensor("oh4", [P, 4, 4], BF16, kind="ExternalInput").ap()
    out = nc.dram_tensor("out", [S, H], BF16, kind="ExternalOutput").ap()

    hT3 = hT.rearrange("(ko p) s -> p ko s", p=P)
    wqkv3 = wqkv.rearrange("(ko p) c -> p ko c", p=P)
    wo3 = wo.rearrange("(ks p) n -> p ks n", p=P)
    out3 = out.rearrange("(st p) n -> p st n", p=P)

    with ExitStack() as ctx:
        tc = ctx.enter_context(tile.TileContext(nc))
        consts = ctx.enter_context(tc.tile_pool(name="consts", bufs=1))
        qkvp = ctx.enter_context(tc.tile_pool(name="qkvp", bufs=1))
        vtp = ctx.enter_context(tc.tile_pool(name="vtp", bufs=1))
        ps_small = ctx.enter_context(
            tc.tile_pool(name="ps_small", bufs=2, space="PSUM")
        )
        abp = ExitStack()
        csp = abp.enter_context(tc.tile_pool(name="csp", bufs=1))

        # constants on the gpsimd (SWDGE) queue; sync queue starts on
        # weights.  cos/sin rope tables are host-computed (exact) and
        # DMA'd in per-st chunks so the big table transfers don't steal
        # HBM bandwidth from the critical first weight/hidden stream.
        wqn_sb = consts.tile([D, 1], F32)
        nc.gpsimd.dma_start(wqn_sb, wqn)
        wkn_sb = consts.tile([D, 1], F32)
        nc.gpsimd.dma_start(wkn_sb, wkn)
        oh5_sb = consts.tile([P, 5, 5], BF16)
        nc.gpsimd.dma_start(oh5_sb, oh5)
        onesrow_sb = consts.tile([1, P], BF16)
        nc.gpsimd.dma_start(onesrow_sb, onesrow)
        rmat_sb = consts.tile([P, P], BF16)
        nc.gpsimd.dma_start(rmat_sb, rmat)
        cosb = csp.tile([P, S], F32)
        sinb = csp.tile([P, S], F32)
        for ts_ in range(ST):
            tsl = slice(512 * ts_, 512 * (ts_ + 1))
            nc.gpsimd.dma_start(cosb[:, tsl], cosbt[:, tsl])
            nc.gpsimd.dma_start(sinb[:, tsl], sinbt[:, tsl])
        identr_sb = consts.tile([P, P], BF16)
        nc.gpsimd.dma_start(identr_sb, identr)
        maskneg_sb = consts.tile([P, 896], BF16)
        nc.gpsimd.dma_start(maskneg_sb, maskneg)
        oh4_sb = consts.tile([P, 4, 4], BF16)
        nc.gpsimd.dma_start(oh4_sb, oh4)
        eps5_sb = consts.tile([5, 1], F32)
        nc.vector.memset(eps5_sb, EPS)
        zero4_sb = consts.tile([4, 1], F32)
        nc.vector.memset(zero4_sb, 0.0)

        q_sb = qkvp.tile([P, NH_L, S], BF16)
        k_sb = qkvp.tile([P, S], BF16)
        vT_sb = vtp.tile([P, KO, P], BF16)

        wqkv_p = abp.enter_context(tc.tile_pool(name="wqkv_p", bufs=1))
        ht_p = abp.enter_context(tc.tile_pool(name="ht_p", bufs=4))
        cpool = abp.enter_context(tc.tile_pool(name="cpool", bufs=1))

        # ---- Phase A: QKV projection + fused norm/rope/v-transpose ----
        # host layout: wqkv cols = [k(128), v(128), q(512)]
        CT_ORDER = [0, 1, 2, 3, 4, 5]  # k, v, then q heads

        def chunk_stage1(x_ch, w_sb, st, row, ssq5, ps_c):
            """x^2 row into the st-shared ssq psum + w-mult + rope of one
            chunk.  Emitted one QKV group late so the ssq/qR matmuls'
            DVE inputs are ready when PE reaches them.  The rms
            normalize itself is batched per st (see rms_finish)."""
            sl = slice(512 * st, 512 * (st + 1))
            xsq = cpool.tile([P, 512], BF16, tag="ctmp", bufs=4, name="xsq")
            nc.vector.tensor_mul(xsq, x_ch, x_ch)
            nc.tensor.matmul(ssq5, oh5_sb[:, row], xsq,
                             start=(row == 0), stop=(row == 4))
            nc.vector.tensor_scalar_mul(x_ch, x_ch, w_sb)
            t1m = cpool.tile([P, 512], BF16, tag="ctmp", bufs=4, name="t1m")
            nc.vector.tensor_mul(t1m, x_ch, cosb[:, sl])
            qr_ps = ps_c.tile([P, 512], F32, tag="qr", name="qr_ps")
            nc.tensor.matmul(qr_ps, rmat_sb, x_ch, start=True, stop=True)
            nc.vector.tensor_tensor(x_ch, qr_ps, sinb[:, sl], OP.mult)
            nc.vector.tensor_tensor(x_ch, x_ch, t1m, OP.add)

        def chunk_stage2(x_ch, ri5, row, ps_c):
            """normalize by the broadcast 1/rms row (runs late, q/k chunks
            aren't read until attention).  gpsimd partition_broadcast only
            reads partition 0, so rows >0 hop there via a 1KB DMA first;
            everything stays on the idle gpsimd engine."""
            src = ri5[row : row + 1]
            if row > 0:
                rr = cpool.tile([1, 512], BF16, tag="rr", bufs=2, name="rr")
                nc.gpsimd.dma_start(rr, src)
                src = rr
            rb_sb = cpool.tile([P, 512], BF16, tag="rb", bufs=3, name="rb_sb")
            nc.gpsimd.partition_broadcast(rb_sb, src, 128)
            nc.vector.tensor_tensor(x_ch, x_ch, rb_sb, OP.mult)

        with nc.named_scope("qkv_proj"):
            with tc.tile_pool(name="ps_a", bufs=4, space="PSUM") as ps_a, \
                 tc.tile_pool(name="ps_c", bufs=2, space="PSUM") as ps_c:
                wq_sb = wqkv_p.tile([P, KO, CW], BF16)
                d1 = []  # stage-1 closures, one group late
                d2 = []  # stage-2 closures, drained one per group
                for st in range(ST):
                    ssl = slice(512 * st, 512 * (st + 1))
                    hts = []
                    if st == 0:
                        # k/v weight columns stream first (two halves so
                        # the first psum group isn't gated on all 16 ko);
                        # q columns follow the hiddens
                        nc.sync.dma_start(
                            wq_sb[:, 0:8, 0 : 2 * D], wqkv3[:, 0:8, 0 : 2 * D])
                    for g in range(4):  # quarter-K chunks of 4 ko each
                        ht_sb = ht_p.tile([P, KO // 4, 512], BF16, tag="ht",
                                          name=f"ht_{st}_{g}")
                        if st == 0 and g == 0:
                            # split so the first matmul starts sooner
                            nc.sync.dma_start(ht_sb[:, 0:2], hT3[:, 0:2, ssl])
                            nc.sync.dma_start(ht_sb[:, 2:4], hT3[:, 2:4, ssl])
                        else:
                            nc.sync.dma_start(
                                ht_sb, hT3[:, 4 * g : 4 * g + 4, ssl])
                        hts.append(ht_sb)
                        if st == 0 and g == 1:
                            nc.sync.dma_start(
                                wq_sb[:, 8:KO, 0 : 2 * D],
                                wqkv3[:, 8:KO, 0 : 2 * D])
                    if st == 0:
                        half_q = 2 * D + QW // 2
                        nc.sync.dma_start(
                            wq_sb[:, :, 2 * D : half_q],
                            wqkv3[:, :, 2 * D : half_q])
                        nc.sync.dma_start(
                            wq_sb[:, :, half_q:CW], wqkv3[:, :, half_q:CW])
                    ssq5 = ps_small.tile([5, 512], F32, tag="small_ps",
                                         name=f"ssq5_{st}")
                    ri5 = cpool.tile([5, 512], BF16, tag="ri_r", bufs=2,
                                     name=f"ri5_{st}")
                    st_chunks = []
                    for ct in CT_ORDER:
                        acc = ps_a.tile([P, 512], F32, tag="qkv_ps",
                                        name=f"qkv_ps_{st}_{ct}")
                        for ko in range(KO):
                            nc.tensor.matmul(
                                acc,
                                wq_sb[:, ko, P * ct : P * (ct + 1)],
                                hts[ko // 4][:, ko % 4],
                                start=(ko == 0),
                                stop=(ko == KO - 1),
                            )
                        # psum->sbuf copies run on ACT (idle here), keeping
                        # DVE for the rope/rms elementwise chain
                        if ct == 0:
                            x_ch, w_sb, row = k_sb[:, ssl], wkn_sb, 0
                            nc.scalar.copy(x_ch, acc)
                        elif ct == 1:
                            vch = cpool.tile([P, 512], BF16, tag="vch", bufs=2,
                                             name="vch")
                            nc.scalar.copy(vch, acc)
                        else:
                            x_ch, w_sb, row = q_sb[:, ct - 2, ssl], wqn_sb, ct - 1
                            nc.scalar.copy(x_ch, acc)

                        # run pipelined stages of earlier chunks
                        if d1:
                            d1.pop(0)()
                        if d2:
                            d2.pop(0)()

                        if ct == 1:
                            def vtrans(vch=vch, st=st):
                                for i in range(4):
                                    vt_ps = ps_c.tile([P, P], BF16, tag="qr",
                                                      name="vt_ps")
                                    nc.tensor.transpose(
                                        vt_ps, vch[:, P * i : P * (i + 1)],
                                        identr_sb)
                                    nc.vector.tensor_copy(
                                        vT_sb[:, 4 * st + i], vt_ps)
                            d1.append(vtrans)
                        else:
                            st_chunks.append((x_ch, row))

                            def s1(x_ch=x_ch, w_sb=w_sb, st=st, row=row,
                                   ssq5=ssq5):
                                chunk_stage1(x_ch, w_sb, st, row, ssq5, ps_c)
                            d1.append(s1)

                    def rms_finish(ssq5=ssq5, ri5=ri5, chunks=st_chunks):
                        ln5 = cpool.tile([5, 512], F32, tag="ln_r", bufs=2,
                                         name="ln5")
                        nc.scalar.activation(ln5, ssq5, AF.Ln, bias=eps5_sb,
                                             scale=1.0 / D)
                        nc.scalar.activation(ri5, ln5, AF.Exp, scale=-0.5)
                        for x_ch, row in chunks:
                            def s2(x_ch=x_ch, row=row, ri5=ri5):
                                chunk_stage2(x_ch, ri5, row, ps_c)
                            d2.append(s2)
                    d1.append(rms_finish)
                while d1:
                    d1.pop(0)()
                while d2:
                    d2.pop(0)()

        abp.close()  # release cos/sin tables + norm temps

        # ------------- Phase E/F: attention + out projection -------------
        with tc.tile_pool(name="otp", bufs=1) as otp, \
             tc.tile_pool(name="wop", bufs=1) as wop, \
             tc.tile_pool(name="expp", bufs=14) as expp, \
             tc.tile_pool(name="dinvp", bufs=4) as dinvp, \
             tc.tile_pool(name="dbp", bufs=3) as dbp, \
             tc.tile_pool(name="outp", bufs=3) as outp, \
             tc.tile_pool(name="ps_st", bufs=2, space="PSUM") as ps_st, \
             tc.tile_pool(name="ps_o", bufs=2, space="PSUM") as ps_o:
            oT_sb = otp.tile([P, NH_L, S], BF16)
            wo_sb = wop.tile([P, NH_L, H], BF16)
            for ks in range(NH_L):
                nc.sync.dma_start(wo_sb[:, ks], wo3[:, ks])

            carry = []  # deferred den/pv/copy work from the previous head

            def attn_tile(st, hh, den4):
                n_sk = 4 * st + 4
                qh = q_sb[:, hh, 512 * st : 512 * (st + 1)]
                o_ps = ps_o.tile([P, 512], F32, tag="o_ps", name="o_ps")

                def vis0(j):
                    # first visible sq column of sk-block j within this
                    # 512-wide sq tile; columns below it are fully masked
                    # and never computed/read anywhere
                    return max(0, 128 * (j - 4 * st))

                def emit_den(eps):
                    # head hh's denominator accumulates into row hh of the
                    # st-shared [4,512] psum (one-hot stationary), one
                    # accumulation epoch across all four heads
                    for ep, j0 in eps:
                        for u in (0, 1):
                            j = j0 + u
                            c0 = vis0(j)
                            nc.tensor.matmul(den4[:, c0:512], oh4_sb[:, hh],
                                             ep[:, u, c0:512],
                                             start=(hh == 0 and j == 0),
                                             stop=(hh == NH_L - 1
                                                   and j == n_sk - 1))

                def emit_pv(eps):
                    for ep, j0 in eps:
                        for u in (0, 1):
                            j = j0 + u
                            c0 = vis0(j)
                            nc.tensor.matmul(o_ps[:, c0:512], vT_sb[:, j],
                                             ep[:, u, c0:512],
                                             start=(j == 0),
                                             stop=(j == n_sk - 1))

                pend = []
                for m in range(2 * st + 2):
                    sT = ps_st.tile([P, 1024], F32, tag="sT", name="sT")
                    masks = []
                    for u in (0, 1):
                        j = 2 * m + u
                        half = sT[:, 512 * u : 512 * (u + 1)]
                        if j >= 4 * st:
                            # scores over the visible span (start marks the
                            # whole bank); the [128,128] staircase masks of
                            # both halves run after, sharing one stationary
                            c0 = vis0(j)
                            nc.tensor.matmul(
                                half[:, c0:512],
                                k_sb[:, P * j : P * (j + 1)], qh[:, c0:512],
                                start=True, stop=False)
                            masks.append((half, c0))
                        else:
                            nc.tensor.matmul(
                                half, k_sb[:, P * j : P * (j + 1)], qh,
                                start=True, stop=True)
                    for half, c0 in masks:
                        nc.tensor.matmul(
                            half[:, c0 : c0 + 128], identr_sb,
                            maskneg_sb[:, 384:512],
                            start=False, stop=True)
                    ep = expp.tile([P, 2, 512], BF16, tag="ep", name="ep")
                    nc.scalar.activation(ep, sT, AF.Exp, scale=SM_SCALE)
                    # interleave the previous head's deferred den/PV (its
                    # exps are long done) with this head's scores, so PE
                    # never drains a tile's den/PV right after its exps
                    pend.append((ep, 2 * m))
                    if carry:
                        carry.pop(0)()
                    elif len(pend) > 3:
                        batch, pend = pend[:2], pend[2:]
                        emit_den(batch)
                        emit_pv(batch)

                def mk(batch):
                    def go():
                        emit_den(batch)
                        emit_pv(batch)
                    return go

                while pend:
                    batch, pend = pend[:2], pend[2:]
                    carry.append(mk(batch))

                # o psum freed by the deferred copy; normalization is
                # batched per st in the st finisher
                osl = oT_sb[:, hh, 512 * st : 512 * (st + 1)]

                def ocopy(osl=osl, o_ps=o_ps):
                    nc.vector.tensor_copy(osl, o_ps)
                carry.append(ocopy)
                return osl

            def out_proj(st):
                # the final tile's out-proj is the serial tail: attention is
                # done, so spread its psum groups over the idle scores slots
                # and its copies over both ACT and DVE
                tail = st == ST - 1
                with nc.named_scope(f"out_proj_t{st}"):
                    for t in range(4 * st, 4 * st + 4):
                        out_sb = outp.tile([P, H], BF16, tag="out_sb",
                                           name="out_sb")
                        for nt in range(4):
                            if tail and nt % 2 == 1:
                                acc = ps_st.tile([P, 512], F32, tag="sT",
                                                 name="out_ps_b")
                            else:
                                acc = ps_o.tile([P, 512], F32, tag="o_ps",
                                                name="out_ps")
                            for ks in range(NH_L):
                                nc.tensor.matmul(
                                    acc,
                                    oT_sb[:, ks, P * t : P * (t + 1)],
                                    wo_sb[:, ks, 512 * nt : 512 * (nt + 1)],
                                    start=(ks == 0),
                                    stop=(ks == NH_L - 1),
                                )
                            osl = out_sb[:, 512 * nt : 512 * (nt + 1)]
                            if tail and nt % 2 == 1:
                                nc.scalar.copy(osl, acc)
                            else:
                                nc.vector.tensor_copy(osl, acc)
                            if tail:
                                # chunked store so the last DMAs don't all
                                # trail the final copy
                                nc.sync.dma_start(
                                    out3[:, t, 512 * nt : 512 * (nt + 1)], osl)
                        if not tail:
                            nc.sync.dma_start(out3[:, t], out_sb)

            for st in range(ST):
                den4 = ps_small.tile([4, 512], F32, tag="small_ps",
                                     name=f"den4_{st}")
                osls = []
                for hh in range(NH_L):
                    with nc.named_scope(f"attn_h{hh}_t{st}"):
                        osls.append(attn_tile(st, hh, den4))

                def fin_st(st=st, den4=den4, osls=osls):
                    # one Ln/Exp pair covers all four heads' denominators
                    ln4 = dinvp.tile([4, 512], F32, tag="ln_r", name="ln4")
                    nc.scalar.activation(ln4, den4, AF.Ln, bias=zero4_sb)
                    di4 = dinvp.tile([4, 512], BF16, tag="di_r", name="di4")
                    nc.scalar.activation(di4, ln4, AF.Exp, scale=-1.0)
                    for hh in range(NH_L):
                        src = di4[hh : hh + 1]
                        if hh > 0:
                            dr = dbp.tile([1, 512], BF16, tag="dr", name="dr")
                            nc.gpsimd.dma_start(dr, src)
                            src = dr
                        db_sb = dbp.tile([P, 512], BF16, tag="db",
                                         name="db_sb")
                        nc.gpsimd.partition_broadcast(db_sb, src, 128)
                        nc.vector.tensor_tensor(osls[hh], osls[hh], db_sb,
                                                OP.mult)
                    out_proj(st)
                carry.append(fin_st)
            while carry:
                carry.pop(0)()

    nc.compile()
    _merge_act_table_loads(nc)
    return nc


def _merge_act_table_loads(nc):
    """Ln(5)/Exp+Copy(0) both live in set 6 (natural_log_exp_and_others);
    bass's per-function table choice alternates 5/0 and reloads tables at
    every Ln<->Exp transition (~1.5us each, 73 loads).  Retarget those
    loads to set 6 and drop the now-redundant reloads."""
    for b in nc.main_func.blocks:
        loaded = None
        keep = []
        for inst in b.instructions:
            if isinstance(inst, mybir.InstLoadActFuncSet):
                tid = inst.act_func_set_id
                if tid in (0, 5):
                    tid = 6
                if tid == loaded:
                    continue
                inst.act_func_set_id = tid
                loaded = tid
            keep.append(inst)
        b.instructions[:] = keep


_NC_CACHE = None


def _get_nc():
    global _NC_CACHE
    if _NC_CACHE is None:
        _NC_CACHE = _build()
    return _NC_CACHE


def _host_inputs(positions, hidden_states, w_qkv, w_o, q_norm_w, k_norm_w):
    """Build the 8 per-core input maps."""
    positions = np.asarray(positions, dtype=np.int32)
    hidden_states = np.asarray(hidden_states, dtype=np.float32)
    w_qkv = np.asarray(w_qkv, dtype=np.float32)
    w_o = np.asarray(w_o, dtype=np.float32)
    q_norm_w = np.asarray(q_norm_w, dtype=np.float32)
    k_norm_w = np.asarray(k_norm_w, dtype=np.float32)

    invf = 1.0 / (ROPE_THETA ** (np.arange(0, D, 2, dtype=np.float64) / D))
    p_idx = np.arange(P).reshape(P, 1)
    c_idx = np.arange(896).reshape(1, 896)
    maskneg = np.where(p_idx > c_idx - 384, np.float32(NEG), np.float32(0.0))
    maskneg = maskneg.astype(BF)
    ones = np.ones((P, 1), dtype=BF)
    onesrow = np.ones((1, P), dtype=BF)
    identr = np.eye(P, dtype=BF)
    rmat = np.zeros((P, P), dtype=BF)
    for i in range(64):
        rmat[64 + i, i] = -1.0
        rmat[i, 64 + i] = 1.0
    oh5 = np.zeros((P, 5, 5), dtype=BF)
    oh4 = np.zeros((P, 4, 4), dtype=BF)
    for r in range(5):
        oh5[:, r, r] = 1.0
    for r in range(4):
        oh4[:, r, r] = 1.0
    wqn = q_norm_w.reshape(D, 1)
    wkn = k_norm_w.reshape(D, 1)

    # host-exact rope tables per batch: row r (r%64 = freq) x position
    cosbt, sinbt = [], []
    for g in range(positions.shape[0]):
        ang = np.outer(invf, positions[g].astype(np.float64))  # [64, S]
        c = np.cos(ang).astype(np.float32)
        s = np.sin(ang).astype(np.float32)
        cosbt.append(np.concatenate([c, c], axis=0))
        sinbt.append(np.concatenate([s, s], axis=0))

    in_maps = []
    for core in range(8):
        g, r = core // 4, core % 4
        wq_cols = w_qkv[:, 512 * r : 512 * (r + 1)]
        wk_col = w_qkv[:, 2048 + 128 * r : 2048 + 128 * (r + 1)]
        wv_col = w_qkv[:, 2560 + 128 * r : 2560 + 128 * (r + 1)]
        in_maps.append(
            {
                "hT": np.ascontiguousarray(hidden_states[g].T).astype(BF),
                "wqkv": np.ascontiguousarray(
                    np.concatenate([wk_col, wv_col, wq_cols], axis=1)
                ).astype(BF),
                "wo": np.ascontiguousarray(
                    w_o[512 * r : 512 * (r + 1), :]
                ).astype(BF),
                "cosbt": cosbt[g],
                "sinbt": sinbt[g],
                "wqn": wqn,
                "wkn": wkn,
                "maskneg": maskneg,
                "onesrow": onesrow,
                "identr": identr,
                "rmat": rmat,
                "oh5": oh5,
                "oh4": oh4,
            }
        )
    return in_maps


def run(trace=False, **inputs):
    nc = _get_nc()
    in_maps = _host_inputs(**inputs)
    res = run_bass_kernel_spmd(nc, in_maps, core_ids=list(range(8)), trace=trace)
    B = inputs["hidden_states"].shape[0]
    out = np.zeros((B, S, H), dtype=np.float64)
    for core in range(8):
        g = core // 4
        out[g] += res.results[core]["out"].astype(np.float64)
    return out.astype(np.float32), res


def kernel(**inputs):
    out, _ = run(trace=False, **inputs)
    return out



# revision 78
# speedup vs baseline: 1.0702x; 1.0702x over previous
"""BaiLing attention Trainium2 kernel.

Sharding: 8 cores = 2 (batch) x 4 (tensor-parallel over heads).
Each TP rank r owns q heads 4r..4r+3 and kv head r (GQA group-aligned),
computes its out-projection partial; host sums the 4 partials per batch.

On-chip layout is feature-major [d, s] everywhere:
  - QKV proj:  psum[qkv_col, s] = w_qkv_slice.T @ hidden.T
  - scoresT:   psum[sk, sq]     = k_tile.T @ q_tile    (both [d, *])
  - causal mask: -1e9 accumulated into the scores psum via an
    identity-matmul of a constant mask tile
  - softmax over sk (partition dim): exp on ACT, denominator via
    ones-matmul, unnormalized PV; 1/x and 1/sqrt(x) of the [1,n]
    rows are computed as Exp(-c*Ln(x)) on ACT (DVE reciprocal on a
    single-partition row is lane-bound and ~25x slower) and broadcast
    across partitions with a K=1 ones-row matmul into PSUM (no DMA
    round-trips).
  - PV:        psum[d, sq]      = vT_tile.T @ probsT
  - out-proj:  psum[s, n]       = oT_slice.T @ w_o_slice
Matmuls run as float32r. Out-proj tiles are emitted one attention tile
late so the tensor engine queue never waits on the normalize chain.
"""

import sys

sys.path.insert(0, "/opt/trn_rl_repo")

import math
from contextlib import ExitStack

import ml_dtypes
import numpy as np

BF = ml_dtypes.bfloat16

import concourse.bass as bass
import concourse.mybir as mybir
import concourse.tile as tile
from concourse import bacc
from concourse.bass_utils import run_bass_kernel_spmd

F32 = mybir.dt.float32
F32R = mybir.dt.float32r
BF16 = mybir.dt.bfloat16
I32 = mybir.dt.int32
AF = mybir.ActivationFunctionType
OP = mybir.AluOpType

H = 2048          # hidden size
S = 2048          # sequence length
D = 128           # head dim
NH_L = 4          # q heads per rank
QW = NH_L * D     # 512 local q width
CW = QW + 2 * D   # 768 local qkv width
P = 128
KO = H // P       # 16 contraction tiles
ST = S // 512     # 4 seq tiles of 512
SM_SCALE = float(D) ** -0.5
EPS = 1e-6
ROPE_THETA = 10000.0
NEG = -1.0e9

# Cody-Waite split of 2*pi (short-mantissa terms: k*ci exact for k<=512)
CW1 = 6.28125
CW2 = 0.0019350051879882812
CW3 = 3.019916050561733e-07
PI = math.pi


def _build():
    nc = bacc.Bacc("TRN2", target_bir_lowering=False, debug=False, num_devices=8)

    hT = nc.dram_tensor("hT", [H, S], BF16, kind="ExternalInput").ap()
    wqkv = nc.dram_tensor("wqkv", [H, CW], BF16, kind="ExternalInput").ap()
    wo = nc.dram_tensor("wo", [QW, H], BF16, kind="ExternalInput").ap()
    cosbt = nc.dram_tensor("cosbt", [P, S], F32, kind="ExternalInput").ap()
    sinbt = nc.dram_tensor("sinbt", [P, S], F32, kind="ExternalInput").ap()
    wqn = nc.dram_tensor("wqn", [D, 1], F32, kind="ExternalInput").ap()
    wkn = nc.dram_tensor("wkn", [D, 1], F32, kind="ExternalInput").ap()
    maskneg = nc.dram_tensor("maskneg", [P, 896], BF16, kind="ExternalInput").ap()
    identr = nc.dram_tensor("identr", [P, P], BF16, kind="ExternalInput").ap()
    rmat = nc.dram_tensor("rmat", [P, P], BF16, kind="ExternalInput").ap()
    oh5 = nc.dram_tensor("oh5", [P, 5, 5], BF16, kind="ExternalInput").ap()
    oh4 = nc.dram_tensor("oh4", [P, 4, 4], BF16, kind="ExternalInput").ap()
    out = nc.dram_tensor("out", [S, H], F32, kind="ExternalOutput").ap()

    hT3 = hT.rearrange("(ko p) s -> p ko s", p=P)
    wqkv3 = wqkv.rearrange("(ko p) c -> p ko c", p=P)
    wo3 = wo.rearrange("(ks p) n -> p ks n", p=P)
    out3 = out.rearrange("(st p) n -> p st n", p=P)

    with ExitStack() as ctx:
        tc = ctx.enter_context(tile.TileContext(nc))
        consts = ctx.enter_context(tc.tile_pool(name="consts", bufs=1))
        qkvp = ctx.enter_context(tc.tile_pool(name="qkvp", bufs=1))
        vtp = ctx.enter_context(tc.tile_pool(name="vtp", bufs=1))
        ps_small = ctx.enter_context(
            tc.tile_pool(name="ps_small", bufs=2, space="PSUM")
        )
        abp = ExitStack()
        csp = abp.enter_context(tc.tile_pool(name="csp", bufs=1))

        # constants on the gpsimd (SWDGE) queue; sync queue starts on
        # weights.  cos/sin rope tables are host-computed (exact) and
        # DMA'd in per-st chunks so the big table transfers don't steal
        # HBM bandwidth from the critical first weight/hidden stream.
        wqn_sb = consts.tile([D, 1], F32)
        nc.gpsimd.dma_start(wqn_sb, wqn)
        wkn_sb = consts.tile([D, 1], F32)
        nc.gpsimd.dma_start(wkn_sb, wkn)
        oh5_sb = consts.tile([P, 5, 5], BF16)
        nc.gpsimd.dma_start(oh5_sb, oh5)
        rmat_sb = consts.tile([P, P], BF16)
        nc.gpsimd.dma_start(rmat_sb, rmat)
        cosb = csp.tile([P, S], F32)
        sinb = csp.tile([P, S], F32)
        for ts_ in range(ST):
            tsl = slice(512 * ts_, 512 * (ts_ + 1))
            nc.gpsimd.dma_start(cosb[:, tsl], cosbt[:, tsl])
            nc.gpsimd.dma_start(sinb[:, tsl], sinbt[:, tsl])
        identr_sb = consts.tile([P, P], BF16)
        nc.gpsimd.dma_start(identr_sb, identr)
        maskneg_sb = consts.tile([P, 896], BF16)
        nc.gpsimd.dma_start(maskneg_sb, maskneg)
        oh4_sb = consts.tile([P, 4, 4], BF16)
        nc.gpsimd.dma_start(oh4_sb, oh4)
        eps5_sb = consts.tile([5, 1], F32)
        nc.vector.memset(eps5_sb, EPS)
        zero4_sb = consts.tile([4, 1], F32)
        nc.vector.memset(zero4_sb, 0.0)

        q_sb = qkvp.tile([P, NH_L, S], BF16)
        k_sb = qkvp.tile([P, S], BF16)
        vT_sb = vtp.tile([P, KO, P], BF16)

        wqkv_p = abp.enter_context(tc.tile_pool(name="wqkv_p", bufs=1))
        ht_p = abp.enter_context(tc.tile_pool(name="ht_p", bufs=4))
        cpool = abp.enter_context(tc.tile_pool(name="cpool", bufs=1))

        # ---- Phase A: QKV projection + fused norm/rope/v-transpose ----
        # host layout: wqkv cols = [k(128), v(128), q(512)]
        CT_ORDER = [0, 1, 2, 3, 4, 5]  # k, v, then q heads

        def chunk_stage1(x_ch, w_sb, st, row, ssq5, ps_c):
            """x^2 row into the st-shared ssq psum + w-mult + rope of one
            chunk.  Emitted one QKV group late so the ssq/qR matmuls'
            DVE inputs are ready when PE reaches them.  The rms
            normalize itself is batched per st (see rms_finish)."""
            sl = slice(512 * st, 512 * (st + 1))
            xsq = cpool.tile([P, 512], BF16, tag="ctmp", bufs=4, name="xsq")
            nc.vector.tensor_mul(xsq, x_ch, x_ch)
            nc.tensor.matmul(ssq5, oh5_sb[:, row], xsq,
                             start=(row == 0), stop=(row == 4))
            nc.vector.tensor_scalar_mul(x_ch, x_ch, w_sb)
            t1m = cpool.tile([P, 512], BF16, tag="ctmp", bufs=4, name="t1m")
            nc.vector.tensor_mul(t1m, x_ch, cosb[:, sl])
            qr_ps = ps_c.tile([P, 512], F32, tag="qr", name="qr_ps")
            nc.tensor.matmul(qr_ps, rmat_sb, x_ch, start=True, stop=True)
            nc.vector.tensor_tensor(x_ch, qr_ps, sinb[:, sl], OP.mult)
            nc.vector.tensor_tensor(x_ch, x_ch, t1m, OP.add)

        def chunk_stage2(x_ch, ri5, row, ps_c):
            """normalize by the broadcast 1/rms row (runs late, q/k chunks
            aren't read until attention).  gpsimd partition_broadcast only
            reads partition 0, so rows >0 hop there via a 1KB DMA first;
            everything stays on the idle gpsimd engine."""
            src = ri5[row : row + 1]
            if row > 0:
                rr = cpool.tile([1, 512], BF16, tag="rr", bufs=2, name="rr")
                nc.gpsimd.dma_start(rr, src)
                src = rr
            rb_sb = cpool.tile([P, 512], BF16, tag="rb", bufs=3, name="rb_sb")
            nc.gpsimd.partition_broadcast(rb_sb, src, 128)
            nc.vector.tensor_tensor(x_ch, x_ch, rb_sb, OP.mult)

        with nc.named_scope("qkv_proj"):
            with tc.tile_pool(name="ps_a", bufs=4, space="PSUM") as ps_a, \
                 tc.tile_pool(name="ps_c", bufs=2, space="PSUM") as ps_c:
                wq_sb = wqkv_p.tile([P, KO, CW], BF16)
                d1 = []  # stage-1 closures, one group late
                d2 = []  # stage-2 closures, drained one per group
                for st in range(ST):
                    ssl = slice(512 * st, 512 * (st + 1))
                    hts = []
                    if st == 0:
                        # k/v weight columns stream first (two halves so
                        # the first psum group isn't gated on all 16 ko);
                        # q columns follow the hiddens
                        nc.sync.dma_start(
                            wq_sb[:, 0:8, 0 : 2 * D], wqkv3[:, 0:8, 0 : 2 * D])
                    for g in range(4):  # quarter-K chunks of 4 ko each
                        ht_sb = ht_p.tile([P, KO // 4, 512], BF16, tag="ht",
                                          name=f"ht_{st}_{g}")
                        if st == 0 and g == 0:
                            # split so the first matmul starts sooner
                            nc.sync.dma_start(ht_sb[:, 0:2], hT3[:, 0:2, ssl])
                            nc.sync.dma_start(ht_sb[:, 2:4], hT3[:, 2:4, ssl])
                        else:
                            nc.sync.dma_start(
                                ht_sb, hT3[:, 4 * g : 4 * g + 4, ssl])
                        hts.append(ht_sb)
                        if st == 0 and g == 1:
                            nc.sync.dma_start(
                                wq_sb[:, 8:KO, 0 : 2 * D],
                                wqkv3[:, 8:KO, 0 : 2 * D])
                    if st == 0:
                        half_q = 2 * D + QW // 2
                        nc.sync.dma_start(
                            wq_sb[:, :, 2 * D : half_q],
                            wqkv3[:, :, 2 * D : half_q])
                        nc.sync.dma_start(
                            wq_sb[:, :, half_q:CW], wqkv3[:, :, half_q:CW])
                    ssq5 = ps_small.tile([5, 512], F32, tag="small_ps",
                                         name=f"ssq5_{st}")
                    ri5 = cpool.tile([5, 512], BF16, tag="ri_r", bufs=2,
                                     name=f"ri5_{st}")
                    st_chunks = []
                    for ct in CT_ORDER:
                        acc = ps_a.tile([P, 512], F32, tag="qkv_ps",
                                        name=f"qkv_ps_{st}_{ct}")
                        for ko in range(KO):
                            nc.tensor.matmul(
                                acc,
                                wq_sb[:, ko, P * ct : P * (ct + 1)],
                                hts[ko // 4][:, ko % 4],
                                start=(ko == 0),
                                stop=(ko == KO - 1),
                            )
                        # psum->sbuf copies run on ACT (idle here), keeping
                        # DVE for the rope/rms elementwise chain
                        if ct == 0:
                            x_ch, w_sb, row = k_sb[:, ssl], wkn_sb, 0
                            nc.scalar.copy(x_ch, acc)
                        elif ct == 1:
                            vch = cpool.tile([P, 512], BF16, tag="vch", bufs=2,
                                             name="vch")
                            nc.scalar.copy(vch, acc)
                        else:
                            x_ch, w_sb, row = q_sb[:, ct - 2, ssl], wqn_sb, ct - 1
                            nc.scalar.copy(x_ch, acc)

                        # run pipelined stages of earlier chunks
                        if d1:
                            d1.pop(0)()
                        if d2:
                            d2.pop(0)()

                        if ct == 1:
                            def vtrans(vch=vch, st=st):
                                for i in range(4):
                                    vt_ps = ps_c.tile([P, P], BF16, tag="qr",
                                                      name="vt_ps")
                                    nc.tensor.transpose(
                                        vt_ps, vch[:, P * i : P * (i + 1)],
                                        identr_sb)
                                    nc.vector.tensor_copy(
                                        vT_sb[:, 4 * st + i], vt_ps)
                            d1.append(vtrans)
                        else:
                            st_chunks.append((x_ch, row))

                            def s1(x_ch=x_ch, w_sb=w_sb, st=st, row=row,
                                   ssq5=ssq5):
                                chunk_stage1(x_ch, w_sb, st, row, ssq5, ps_c)
                            d1.append(s1)

                    def rms_finish(ssq5=ssq5, ri5=ri5, chunks=st_chunks):
                        ln5 = cpool.tile([5, 512], F32, tag="ln_r", bufs=2,
                                         name="ln5")
                        nc.scalar.activation(ln5, ssq5, AF.Ln, bias=eps5_sb,
                                             scale=1.0 / D)
                        nc.scalar.activation(ri5, ln5, AF.Exp, scale=-0.5)
                        for x_ch, row in chunks:
                            def s2(x_ch=x_ch, row=row, ri5=ri5):
                                chunk_stage2(x_ch, ri5, row, ps_c)
                            d2.append(s2)
                    d1.append(rms_finish)
                while d1:
                    d1.pop(0)()
                while d2:
                    d2.pop(0)()

        abp.close()  # release cos/sin tables + norm temps

        # ------------- Phase E/F: attention + out projection -------------
        with tc.tile_pool(name="otp", bufs=1) as otp, \
             tc.tile_pool(name="wop", bufs=1) as wop, \
             tc.tile_pool(name="expp", bufs=12) as expp, \
             tc.tile_pool(name="dinvp", bufs=4) as dinvp, \
             tc.tile_pool(name="dbp", bufs=3) as dbp, \
             tc.tile_pool(name="outp", bufs=3) as outp, \
             tc.tile_pool(name="ps_st", bufs=2, space="PSUM") as ps_st, \
             tc.tile_pool(name="ps_o", bufs=2, space="PSUM") as ps_o:
            oT_sb = otp.tile([P, NH_L, S], BF16)
            wo_sb = wop.tile([P, NH_L, H], BF16)
            for ks in range(NH_L):
                nc.sync.dma_start(wo_sb[:, ks], wo3[:, ks])

            carry = []  # deferred den/pv/copy work from the previous head

            def attn_tile(st, hh, den4):
                n_sk = 4 * st + 4
                qh = q_sb[:, hh, 512 * st : 512 * (st + 1)]
                o_ps = ps_o.tile([P, 512], F32, tag="o_ps", name="o_ps")

                def vis0(j):
                    # first visible sq column of sk-block j within this
                    # 512-wide sq tile; columns below it are fully masked
                    # and never computed/read anywhere
                    return max(0, 128 * (j - 4 * st))

                def emit_den(eps):
                    # head hh's denominator accumulates into row hh of the
                    # st-shared [4,512] psum (one-hot stationary), one
                    # accumulation epoch across all four heads
                    for ep, j0 in eps:
                        for u in (0, 1):
                            j = j0 + u
                            c0 = vis0(j)
                            nc.tensor.matmul(den4[:, c0:512], oh4_sb[:, hh],
                                             ep[:, u, c0:512],
                                             start=(hh == 0 and j == 0),
                                             stop=(hh == NH_L - 1
                                                   and j == n_sk - 1))

                def emit_pv(eps):
                    for ep, j0 in eps:
                        for u in (0, 1):
                            j = j0 + u
                            c0 = vis0(j)
                            nc.tensor.matmul(o_ps[:, c0:512], vT_sb[:, j],
                                             ep[:, u, c0:512],
                                             start=(j == 0),
                                             stop=(j == n_sk - 1))

                pend = []
                for m in range(2 * st + 2):
                    sT = ps_st.tile([P, 1024], F32, tag="sT", name="sT")
                    for u in (0, 1):
                        j = 2 * m + u
                        half = sT[:, 512 * u : 512 * (u + 1)]
                        if j >= 4 * st:
                            # scores over the visible span (start marks the
                            # whole bank), then the [128,128] staircase mask
                            # accumulated on the diagonal block only
                            c0 = vis0(j)
                            nc.tensor.matmul(
                                half[:, c0:512],
                                k_sb[:, P * j : P * (j + 1)], qh[:, c0:512],
                                start=True, stop=False)
                            nc.tensor.matmul(
                                half[:, c0 : c0 + 128], identr_sb,
                                maskneg_sb[:, 384:512],
                                start=False, stop=True)
                        else:
                            nc.tensor.matmul(
                                half, k_sb[:, P * j : P * (j + 1)], qh,
                                start=True, stop=True)
                    ep = expp.tile([P, 2, 512], BF16, tag="ep", name="ep")
                    nc.scalar.activation(ep, sT, AF.Exp, scale=SM_SCALE)
                    # interleave the previous head's deferred den/PV (its
                    # exps are long done) with this head's scores, so PE
                    # never drains a tile's den/PV right after its exps
                    pend.append((ep, 2 * m))
                    if carry:
                        carry.pop(0)()
                    elif len(pend) > 3:
                        batch, pend = pend[:2], pend[2:]
                        emit_den(batch)
                        emit_pv(batch)

                def mk(batch):
                    def go():
                        emit_den(batch)
                        emit_pv(batch)
                    return go

                while pend:
                    batch, pend = pend[:2], pend[2:]
                    carry.append(mk(batch))

                # o psum freed by the deferred copy; normalization is
                # batched per st in the st finisher
                osl = oT_sb[:, hh, 512 * st : 512 * (st + 1)]

                def ocopy(osl=osl, o_ps=o_ps):
                    nc.vector.tensor_copy(osl, o_ps)
                carry.append(ocopy)
                return osl

            def out_proj(st):
                # the final tile's out-proj is the serial tail: attention is
                # done, so spread its psum groups over the idle scores slots
                # and its copies over both ACT and DVE
                tail = st == ST - 1
                with nc.named_scope(f"out_proj_t{st}"):
                    for t in range(4 * st, 4 * st + 4):
                        out_sb = outp.tile([P, H], F32, tag="out_sb",
                                           name="out_sb")
                        for nt in range(4):
                            if tail and nt % 2 == 1:
                                acc = ps_st.tile([P, 512], F32, tag="sT",
                                                 name="out_ps_b")
                            else:
                                acc = ps_o.tile([P, 512], F32, tag="o_ps",
                                                name="out_ps")
                            for ks in range(NH_L):
                                nc.tensor.matmul(
                                    acc,
                                    oT_sb[:, ks, P * t : P * (t + 1)],
                                    wo_sb[:, ks, 512 * nt : 512 * (nt + 1)],
                                    start=(ks == 0),
                                    stop=(ks == NH_L - 1),
                                )
                            osl = out_sb[:, 512 * nt : 512 * (nt + 1)]
                            if tail and nt % 2 == 1:
                                nc.scalar.copy(osl, acc)
                            else:
                                nc.vector.tensor_copy(osl, acc)
                            if tail:
                                # chunked store so the last DMAs don't all
                                # trail the final copy
                                nc.sync.dma_start(
                                    out3[:, t, 512 * nt : 512 * (nt + 1)], osl)
                        if not tail:
                            nc.sync.dma_start(out3[:, t], out_sb)

            for st in range(ST):
                den4 = ps_small.tile([4, 512], F32, tag="small_ps",
                                     name=f"den4_{st}")
                osls = []
                for hh in range(NH_L):
                    with nc.named_scope(f"attn_h{hh}_t{st}"):
                        osls.append(attn_tile(st, hh, den4))

                def fin_st(st=st, den4=den4, osls=osls):
                    # one Ln/Exp pair covers all four heads' denominators
                    ln4 = dinvp.tile([4, 512], F32, tag="ln_r", name="ln4")
                    nc.scalar.activation(ln4, den4, AF.Ln, bias=zero4_sb)
                    di4 = dinvp.tile([4, 512], BF16, tag="di_r", name="di4")
                    nc.scalar.activation(di4, ln4, AF.Exp, scale=-1.0)
                    for hh in range(NH_L):
                        src = di4[hh : hh + 1]
                        if hh > 0:
                            dr = dbp.tile([1, 512], BF16, tag="dr", name="dr")
                            nc.gpsimd.dma_start(dr, src)
                            src = dr
                        db_sb = dbp.tile([P, 512], BF16, tag="db",
                                         name="db_sb")
                        nc.gpsimd.partition_broadcast(db_sb, src, 128)
                        nc.vector.tensor_tensor(osls[hh], osls[hh], db_sb,
                                                OP.mult)
                    out_proj(st)
                carry.append(fin_st)
            while carry:
                carry.pop(0)()

    nc.compile()
    _merge_act_table_loads(nc)
    return nc


def _merge_act_table_loads(nc):
    """Ln(5)/Exp+Copy(0) both live in set 6 (natural_log_exp_and_others);
    bass's per-function table choice alternates 5/0 and reloads tables at
    every Ln<->Exp transition (~1.5us each, 73 loads).  Retarget those
    loads to set 6 and drop the now-redundant reloads."""
    for b in nc.main_func.blocks:
        loaded = None
        keep = []
        for inst in b.instructions:
            if isinstance(inst, mybir.InstLoadActFuncSet):
                tid = inst.act_func_set_id
                if tid in (0, 5):
                    tid = 6
                if tid == loaded:
                    continue
                inst.act_func_set_id = tid
                loaded = tid
            keep.append(inst)
        b.instructions[:] = keep


_NC_CACHE = None


def _get_nc():
    global _NC_CACHE
    if _NC_CACHE is None:
        _NC_CACHE = _build()
    return _NC_CACHE


def _host_inputs(positions, hidden_states, w_qkv, w_o, q_norm_w, k_norm_w):
    """Build the 8 per-core input maps."""
    positions = np.asarray(positions, dtype=np.int32)
    hidden_states = np.asarray(hidden_states, dtype=np.float32)
    w_qkv = np.asarray(w_qkv, dtype=np.float32)
    w_o = np.asarray(w_o, dtype=np.float32)
    q_norm_w = np.asarray(q_norm_w, dtype=np.float32)
    k_norm_w = np.asarray(k_norm_w, dtype=np.float32)

    invf = 1.0 / (ROPE_THETA ** (np.arange(0, D, 2, dtype=np.float64) / D))
    p_idx = np.arange(P).reshape(P, 1)
    c_idx = np.arange(896).reshape(1, 896)
    maskneg = np.where(p_idx > c_idx - 384, np.float32(NEG), np.float32(0.0))
    maskneg = maskneg.astype(BF)
    ones = np.ones((P, 1), dtype=BF)
    onesrow = np.ones((1, P), dtype=BF)
    identr = np.eye(P, dtype=BF)
    rmat = np.zeros((P, P), dtype=BF)
    for i in range(64):
        rmat[64 + i, i] = -1.0
        rmat[i, 64 + i] = 1.0
    oh5 = np.zeros((P, 5, 5), dtype=BF)
    oh4 = np.zeros((P, 4, 4), dtype=BF)
    for r in range(5):
        oh5[:, r, r] = 1.0
    for r in range(4):
        oh4[:, r, r] = 1.0
    wqn = q_norm_w.reshape(D, 1)
    wkn = k_norm_w.reshape(D, 1)

    # host-exact rope tables per batch: row r (r%64 = freq) x position
    cosbt, sinbt = [], []
    for g in range(positions.shape[0]):
        ang = np.outer(invf, positions[g].astype(np.float64))  # [64, S]
        c = np.cos(ang).astype(np.float32)
        s = np.sin(ang).astype(np.float32)
        cosbt.append(np.concatenate([c, c], axis=0))
        sinbt.append(np.concatenate([s, s], axis=0))

    in_maps = []
    for core in range(8):
        g, r = core // 4, core % 4
        wq_cols = w_qkv[:, 512 * r : 512 * (r + 1)]
        wk_col = w_qkv[:, 2048 + 128 * r : 2048 + 128 * (r + 1)]
        wv_col = w_qkv[:, 2560 + 128 * r : 2560 + 128 * (r + 1)]
        in_maps.append(
            {
                "hT": np.ascontiguousarray(hidden_states[g].T).astype(BF),
                "wqkv": np.ascontiguousarray(
                    np.concatenate([wk_col, wv_col, wq_cols], axis=1)
                ).astype(BF),
                "wo": np.ascontiguousarray(
                    w_o[512 * r : 512 * (r + 1), :]
                ).astype(BF),
                "cosbt": cosbt[g],
                "sinbt": sinbt[g],
                "wqn": wqn,
                "wkn": wkn,
                "maskneg": maskneg,
                "identr": identr,
                "rmat": rmat,
                "oh5": oh5,
                "oh4": oh4,
            }
        )
    return in_maps


def run(trace=False, **inputs):
    nc = _get_nc()
    in_maps = _host_inputs(**inputs)
    res = run_bass_kernel_spmd(nc, in_maps, core_ids=list(range(8)), trace=trace)
    B = inputs["hidden_states"].shape[0]
    out = np.zeros((B, S, H), dtype=np.float64)
    for core in range(8):
        g = core // 4
        out[g] += res.results[core]["out"].astype(np.float64)
    return out.astype(np.float32), res


def kernel(**inputs):
    out, _ = run(trace=False, **inputs)
    return out

